# revision 1
# baseline (speedup 1.0000x reference)
"""Trainium2 Bass kernel for nn_AdjacencyMatrix (gnn_message_passing).

Reference computation:
    m = pad(x, [N, 1024]); repeat num_steps: m = 0.9 * (m @ W)
    y = m[:, -128:] * diag(W)[-128:]

Key algebraic collapse: only the first 256 columns of the padded state are
nonzero and only the last 128 output columns are read, so

    y = 0.9^k * x @ B,   B = (W^k)[0:256, -128:] * diag(W)[-128:]   (per col)

B is computed on-chip via the transposed chain T_i = ((W^i)[0:256, :]).T,
which uses W directly as the matmul stationary operand (no W transpose):

    T_1 = (W[0:256, :]).T            (16 PE tile transposes)
    T_{i+1} = W.T @ T_i              (f32r matmuls, 256-wide moving operand)
    T_k only needs row-tile 7 (cols 896:1024 of W^k)
    B = (diag-scaled T_k).T          (2 PE tile transposes)

Final: y = x @ B via PE with x transposed on-chip (f32r tile transposes).

Sharding: data-parallel over the batch dim N=16384 across 8 cores (2048 rows
per core); W replicated; no collectives.

Schedule notes:
  - DMA loads are chained W -> x (sync deps) so each transfer gets full HBM
    bandwidth and W (the chain's critical input) lands first; T_2 partial
    sums start as soon as each W row-tile arrives.
  - A burst of dummy PE transposes at t=0 trips the PE_HAM activity monitor
    so the real matmuls run at 2.4 GHz instead of the cold 1.2 GHz.
  - PSUM evictions are bank-wide and alternate between DVE and ACT.
"""
import numpy as np

import concourse.bass as bass
import concourse.tile as tile
from concourse import bacc, mybir
from concourse.bass import _add_dep_helper
from concourse.bass_utils import run_bass_kernel_spmd
from concourse.masks import make_identity

F32 = mybir.dt.float32
F32R = mybir.dt.float32r

P = 128
N_ROWS = 16384
N_CORES = 8
ROWS_PER_CORE = N_ROWS // N_CORES  # 2048
D_IN = 256
N_NEURONS = 1024
N_OUT = 128
ENERGY_SCALAR = 0.9

NT = N_NEURONS // P  # 8 row/col tiles of W
DT = D_IN // P  # 2
BT = ROWS_PER_CORE // P  # 16 batch tiles per core

N_WARMUP = 40  # dummy PE ops to trip the HAM clock gate (~3.4us of activity)


def build(num_steps: int) -> "bacc.Bacc":
    assert num_steps >= 1
    nc = bacc.Bacc("TRN2", target_bir_lowering=False, debug=False)

    x_d = nc.dram_tensor("x", [ROWS_PER_CORE, D_IN], F32R, kind="ExternalInput").ap()
    w_d = nc.dram_tensor(
        "weight", [N_NEURONS, N_NEURONS], F32R, kind="ExternalInput"
    ).ap()
    out_d = nc.dram_tensor(
        "out", [ROWS_PER_CORE, N_OUT], F32, kind="ExternalOutput"
    ).ap()

    # alternate PSUM evictions across the two elementwise engines
    _ev = [0]

    def evict(out_ap, in_ap):
        _ev[0] += 1
        if _ev[0] % 2:
            nc.vector.tensor_copy(out_ap, in_ap)
        else:
            nc.scalar.copy(out_ap, in_ap)

    with tile.TileContext(nc) as tc:
        with tc.tile_pool(name="persist", bufs=1) as pp, \
             tc.tile_pool(name="tp_ps", bufs=3, space="PSUM") as tp_ps, \
             tc.tile_pool(name="mm_ps", bufs=3, space="PSUM") as mm_ps, \
             tc.tile_pool(name="y_ps", bufs=2, space="PSUM") as y_ps:
            # identity first: it gates every PE transpose (incl. warm-up)
            ident_f = pp.tile([P, P], F32)
            make_identity(nc, ident_f)
            ident_r = pp.tile([P, P], F32R)
            nc.vector.tensor_copy(ident_r[:], ident_f[:])

            # ---- PE warm-up: dummy transposes, results never read ----
            def dummy_burst(n):
                for _ in range(n // 4):
                    ps = y_ps.tile([P, 4, N_OUT], F32R, tag="y")
                    for i in range(4):
                        nc.tensor.transpose(ps[:, i, :], ident_r[:], ident_r[:])

            dummy_burst(N_WARMUP)

            # ---- input DMAs (HWDGE, f32r end-to-end), chained W -> x ----
            # One dma_start only sustains ~190 GB/s, so run TWO chained load
            # streams in parallel (one issued from SP, one from ACT) with W
            # (the chain's critical input) fully ahead of x; x comes in four
            # fine-grained chunks so transposes/y start per-chunk.
            w_sb = pp.tile([P, NT, N_NEURONS], F32R)
            x_sb = pp.tile([P, BT, D_IN], F32R)

            def w_load(eng, h):
                return eng.dma_start(
                    out=w_sb[:, 2 * h : 2 * h + 2, :],
                    in_=w_d[256 * h : 256 * (h + 1), :].rearrange(
                        "(t p) j -> p t j", p=P
                    ),
                )

            def x_load(eng, q):
                return eng.dma_start(
                    out=x_sb[:, 4 * q : 4 * q + 4, :],
                    in_=x_d[512 * q : 512 * (q + 1), :].rearrange(
                        "(t p) d -> p t d", p=P
                    ),
                )

            # Two chained load streams (one from SP, one from ACT) with W
            # (the chain's critical input) fully ahead of x; x in four
            # fine-grained chunks so transposes/y start per-chunk.
            chains = [
                [w_load(nc.sync, 0), w_load(nc.sync, 2),
                 x_load(nc.sync, 0), x_load(nc.sync, 1)],
                [w_load(nc.scalar, 1), w_load(nc.scalar, 3),
                 x_load(nc.scalar, 2), x_load(nc.scalar, 3)],
            ]
            for chain in chains:
                for a, b in zip(chain[1:], chain[:-1]):
                    _add_dep_helper(a.ins, b.ins, sync=True, reason="load order")

            # diag(W)[-128:] -> [128, 1], scaled by 0.9^k (tiny, off the
            # critical path; needed only at the last chain step)
            diag_raw = pp.tile([P, 1], F32R)
            diag_ap = bass.AP(
                tensor=w_d.tensor,
                offset=(N_NEURONS - N_OUT) * N_NEURONS + (N_NEURONS - N_OUT),
                ap=[[N_NEURONS + 1, P], [1, 1]],
            )
            nc.scalar.dma_start(out=diag_raw[:], in_=diag_ap)
            diag_sc = pp.tile([P, 1], F32)
            nc.vector.tensor_scalar_mul(
                diag_sc[:], diag_raw[:], float(ENERGY_SCALAR**num_steps)
            )

            # ---- T_1 = (W[0:256, :]).T : [128, 8, 256] ----
            # T_1[:, j, 128t:128(t+1)] = (W_sb[:, t, 128j:128(j+1)]).T
            def make_T1(j_tiles, dst):
                for gi in range(0, len(j_tiles), 2):  # 2 j-tiles per bank
                    js = j_tiles[gi : gi + 2]
                    ps = tp_ps.tile([P, DT, DT, P], F32R, tag="tp")
                    for jj, j in enumerate(js):
                        for t in range(DT):
                            nc.tensor.transpose(
                                ps[:, jj, t, :],
                                w_sb[:, t, P * j : P * (j + 1)],
                                ident_r[:],
                            )
                    evict(
                        dst[:, gi : gi + len(js), :].rearrange(
                            "p j (t c) -> p j t c", t=DT
                        ),
                        ps[:, : len(js), :, :],
                    )

            # ---- chain ----
            # T_{i+1}[:, j, :] = sum_a (W_sb[:, a, 128j:]).T @ T_i[:, a, :]
            # NOTE: start=True clears has_written for the WHOLE bank, so each
            # accumulation group needs its own PSUM bank. Interleave the
            # a-loop across a group of 2 banks so partial sums of several
            # j-tiles advance together as W row-tiles arrive from HBM.
            def chain_step(src, j_tiles, dst, scaled=False, after_group=None):
                for gn, gi in enumerate(range(0, len(j_tiles), 2)):
                    js = j_tiles[gi : gi + 2]
                    pss = [
                        mm_ps.tile([P, D_IN], F32, tag="mm", name=f"mm{jj}")
                        for jj in range(len(js))
                    ]
                    a_order = list(range(NT))
                    if gn % 2 == 1:
                        # start on the other load stream's first W chunk so
                        # half the groups make progress whichever DMA stream
                        # wins the race
                        a_order = a_order[2:4] + a_order[0:2] + a_order[6:8] + a_order[4:6]
                    for an, a in enumerate(a_order):
                        for jj, j in enumerate(js):
                            nc.tensor.matmul(
                                pss[jj][:],
                                lhsT=w_sb[:, a, P * j : P * (j + 1)],
                                rhs=src[:, a, :],
                                start=(an == 0),
                                stop=(an == NT - 1),
                            )
                    for jj in range(len(js)):
                        if scaled:
                            # last step: scale by diag * 0.9^k (per-part. c)
                            nc.vector.tensor_scalar_mul(
                                dst[:, gi + jj, :], pss[jj][:], diag_sc[:]
                            )
                        else:
                            evict(dst[:, gi + jj, :], pss[jj][:])
                    if after_group is not None:
                        after_group(gn)

            # ---- x transposes, woven into the chain so their PSUM
            # evictions hide under chain matmuls ----
            xT = pp.tile([P, DT, ROWS_PER_CORE], F32R)
            xt_emitted = set()

            def emit_xT_group(bq):
                if bq in xt_emitted or bq >= BT // 2:
                    return
                xt_emitted.add(bq)
                ps = tp_ps.tile([P, DT, DT, P], F32R, tag="tp", name="xtp")
                for bb in range(2):
                    bt = 2 * bq + bb
                    for v in range(DT):
                        nc.tensor.transpose(
                            ps[:, v, bb, :],
                            x_sb[:, bt, P * v : P * (v + 1)],
                            ident_r[:],
                        )
                evict(
                    xT[:, :, 256 * bq : 256 * (bq + 1)].rearrange(
                        "p v (b c) -> p v b c", b=2
                    ),
                    ps[:, :, :, :],
                )

            def weave(gn):
                emit_xT_group(2 * gn)
                emit_xT_group(2 * gn + 1)

            T4 = pp.tile([P, 1, D_IN], F32R)  # scaled T_k row-tile 7
            if num_steps == 1:
                t1_last = pp.tile([P, 1, D_IN], F32R)
                make_T1([NT - 1], t1_last)
                nc.vector.tensor_scalar_mul(T4[:, 0, :], t1_last[:, 0, :], diag_sc[:])
            else:
                T_cur = pp.tile([P, NT, D_IN], F32R, name="T1")
                make_T1(list(range(NT)), T_cur)
                # bridge the PE stall between T_1 (needs only W row-tiles
                # 0-1) and the chain (needs all of W): keeps the HAM clock
                # warm through the tail of the W load
                dummy_burst(8)
                def t2_fill(gn):
                    # dummy filler: T_2's groups stall on the trickling W
                    # load; keep the PE (and its clock gate) busy meanwhile
                    dummy_burst(8)

                for step in range(2, num_steps):
                    T_nxt = pp.tile([P, NT, D_IN], F32R, name=f"T{step}")
                    last_full = step == num_steps - 1
                    chain_step(
                        T_cur, list(range(NT)), T_nxt,
                        after_group=weave if last_full else t2_fill,
                    )
                    T_cur = T_nxt
                dummy_burst(4)
                chain_step(T_cur, [NT - 1], T4, scaled=True)
                dummy_burst(4)
            for bq in range(BT // 2):
                emit_xT_group(bq)
            dummy_burst(4)

            # ---- B = (T4).T : [128, 2, 128] f32r ----
            B_sb = pp.tile([P, DT, N_OUT], F32R)
            ps_b = tp_ps.tile([P, DT, DT, P], F32R, tag="tp")
            for u in range(DT):
                nc.tensor.transpose(
                    ps_b[:, 0, u, :], T4[:, 0, P * u : P * (u + 1)], ident_r[:]
                )
            nc.vector.tensor_copy(B_sb[:, 0, :], ps_b[:, 0, 0, :])
            nc.scalar.copy(B_sb[:, 1, :], ps_b[:, 0, 1, :])
            dummy_burst(4)

            # ---- y[b, c] = sum_v xT[:, v, b].T @ B[:, v, :] ----
            y_sb = pp.tile([P, BT, N_OUT], F32)
            for g in range(4):
                ps = y_ps.tile([P, 4, N_OUT], F32, tag="y")
                for i in range(4):
                    bt = 4 * g + i
                    for v in range(DT):
                        nc.tensor.matmul(
                            ps[:, i, :],
                            lhsT=xT[:, v, P * bt : P * (bt + 1)],
                            rhs=B_sb[:, v, :],
                            start=(v == 0),
                            stop=(v == DT - 1),
                        )
                if g == 3:
                    # last group: split across both engines to shorten the
                    # kernel tail (eviction and store both halve)
                    nc.vector.tensor_copy(
                        y_sb[:, 4 * g : 4 * g + 2, :], ps[:, 0:2, :]
                    )
                    nc.scalar.copy(
                        y_sb[:, 4 * g + 2 : 4 * g + 4, :], ps[:, 2:4, :]
                    )
                    for h in range(2):
                        oeng = nc.sync if h == 0 else nc.scalar
                        lo = 512 * g + 256 * h
                        oeng.dma_start(
                            out=out_d[lo : lo + 256, :].rearrange(
                                "(t p) c -> p t c", p=P
                            ),
                            in_=y_sb[:, 4 * g + 2 * h : 4 * g + 2 * h + 2, :],
                        )
                else:
                    evict(y_sb[:, 4 * g : 4 * g + 4, :], ps[:, :, :])
                    oeng = nc.sync if g % 2 == 0 else nc.scalar
                    oeng.dma_start(
                        out=out_d[512 * g : 512 * (g + 1), :].rearrange(
                            "(t p) c -> p t c", p=P
                        ),
                        in_=y_sb[:, 4 * g : 4 * g + 4, :],
                    )

    nc.compile()
    return nc


_NC_CACHE: dict = {}


def _get_nc(num_steps: int):
    if num_steps not in _NC_CACHE:
        _NC_CACHE[num_steps] = build(num_steps)
    return _NC_CACHE[num_steps]


def kernel(x: np.ndarray, weight: np.ndarray, num_steps) -> np.ndarray:
    k = int(num_steps)
    x = np.ascontiguousarray(x, dtype=np.float32)
    weight = np.ascontiguousarray(weight, dtype=np.float32)
    if k == 0:
        # pad(x)[:, -128:] is all zero (128 <= 1024 - 256)
        return np.zeros((x.shape[0], N_OUT), dtype=np.float32)

    nc = _get_nc(k)
    in_maps = [
        {
            "x": x[i * ROWS_PER_CORE : (i + 1) * ROWS_PER_CORE],
            "weight": weight,
        }
        for i in range(N_CORES)
    ]
    last_err = None
    for attempt in range(3):
        try:
            res = run_bass_kernel_spmd(nc, in_maps, core_ids=list(range(N_CORES)))
            return np.concatenate(
                [res.results[i]["out"] for i in range(N_CORES)], axis=0
            )
        except Exception as e:  # transient device wedges recover on retry
            last_err = e
            import time as _time

            _time.sleep(10)
    raise last_err

